# revision 4
# baseline (speedup 1.0000x reference)
"""Multi-head attention Bass/Tile kernel for Trainium2, 8-core SPMD.

Problem: B=4, Q=K=2048, D=512, H=8 heads (head dim 64), fp32.
  head_q = q @ Wq.T ; head_k = k @ Wk.T ; head_v = v @ Wv.T
  S = (head_q . head_k) / 8 ; masked softmax over keys ; out = (P . head_v) @ Wo.T

Sharding: data-parallel over (batch, query-half): core c handles batch c//2,
query rows (c%2)*1024 .. +1024.  Each core computes a disjoint output slice;
no collectives.

Host-side prep (per core): masked kv rows are dropped (softmax over keys is
order-invariant and fully-masked rows contribute exactly zero), survivors are
packed into a KLE-row buffer where KLE = max unmasked count over batches
rounded up to 128 (compiled per-KLE, cached).  q/k/v and all weights are
shipped pre-transposed (d-major), so the kernel does no on-chip transposes.

On-core layout:
  - projections contract over d: stationary = W^T chunk [128 din, 128 dout],
    moving = x^T [128 din, cols];  KT/QT come out feature-major, V row-major.
  - scores are computed in S^T[j, i] layout (keys on partitions); head pairs
    share PSUM partition halves so one wide exp covers both.
  - exp() on ScalarE with fused scale+bias (softmax-shift-invariant bias
    keeps the Ln/Exp denominators in range; no max-subtraction needed since
    |score| is bounded ~60).
  - mask multiplies V rows and an extra mask column in the PV stationary
    yields the softmax denominator at PSUM partition 64 for free.
  - normalization: r = exp(-ln(denom)) broadcast to 64 partitions with a
    K=1 matmul against ones, then one DVE multiply into the packed A2 tile
    (head pair on 128 partitions) feeding a K=128 Wo projection.
"""

import sys

if "/opt/trn_rl_repo" not in sys.path:
    sys.path.insert(0, "/opt/trn_rl_repo")

from contextlib import ExitStack

import numpy as np

import concourse.bass as bass
import concourse.tile as tile
from concourse import mybir
import bass_rust as _bass_rust

F32 = mybir.dt.float32
F32R = mybir.dt.float32r
BF16 = mybir.dt.bfloat16
EXP = mybir.ActivationFunctionType.Exp
LN = mybir.ActivationFunctionType.Ln

B, Q, KL, D, H = 4, 2048, 2048, 512, 8
HD = D // H            # 64
QS = Q // 2            # 1024 query rows per core
SCALE = 1.0 / HD ** 0.5
# constant shift inside exp: softmax-invariant, keeps denominators in ACT's
# Ln/Exp accurate range (scores here are ~N(0, 8^2), max |s| ~ 50)
EXPBIAS = -30.0


def _legalize_waits(nc, max_waits=1):
    """This walrus build only encodes one sem-wait per instruction; Tile's
    tail drain carries several.  Split extras onto preceding NoOps."""
    n = 0
    for f in nc.m.functions:
        for bb in f.blocks:
            insts = bb.instructions
            i = 0
            while i < len(insts):
                inst = insts[i]
                si = inst.sync_info
                if si is not None and len(si.on_wait) > max_waits:
                    waits = list(si.on_wait)
                    for j, w in enumerate(waits[max_waits:]):
                        nop = mybir.InstNoOp(
                            name=f"{inst.name}-waitsplit{j}", ins=[], outs=[]
                        )
                        nop.engine = inst.engine
                        nop.sync_info = _bass_rust.SyncInfo(on_wait=[w], on_update=[])
                        insts.insert(i, nop)
                        i += 1
                        n += 1
                    inst.sync_info = _bass_rust.SyncInfo(
                        on_wait=waits[:max_waits], on_update=list(si.on_update)
                    )
                i += 1
    return n


def _r(ap):
    return ap.bitcast(F32R)


def build_kernel(KLE):
    NJT = KLE // 128
    nc = bass.Bass("TRN2", target_bir_lowering=False, debug=False)

    qT_d = nc.dram_tensor("qT", [D, QS], F32R, kind="ExternalInput").ap()
    kT_d = nc.dram_tensor("kT", [D, KLE], F32R, kind="ExternalInput").ap()
    vT_d = nc.dram_tensor("vT", [D, KLE], F32R, kind="ExternalInput").ap()
    w_d = {
        w: nc.dram_tensor(w, [D, D], F32R, kind="ExternalInput").ap()
        for w in ("wqT", "wkT", "wvT", "woT")
    }
    # mask2d[p, t] = float(attn_mask[t*128 + p] != 0) in packed kv order
    m_d = nc.dram_tensor("mask2d", [128, NJT], F32, kind="ExternalInput").ap()
    out_d = nc.dram_tensor("out", [QS, D], F32, kind="ExternalOutput").ap()

    with tile.TileContext(nc) as tc, ExitStack() as ctx:
        # ---- persistent pools -------------------------------------------
        pc = ctx.enter_context(tc.tile_pool(name="const", bufs=1))
        ones_f = pc.tile([128, HD], F32, tag="ones_f")
        nc.vector.memset(ones_f[:], 1.0)
        m_sb = pc.tile([128, NJT], F32, tag="m_sb")
        nc.sync.dma_start(m_sb[:], m_d)
        ebias = pc.tile([128, 1], F32, tag="ebias")
        nc.vector.memset(ebias[:], EXPBIAS)

        # ---- input tiles (DMA direct, pre-transposed on host) -----------
        pin = ctx.enter_context(tc.tile_pool(name="inputs", bufs=1))
        wsb = {}
        for w in ("wkT", "wqT", "wvT", "woT"):
            wsb[w] = [pin.tile([128, D], F32R, tag=f"{w}{i}", name=f"{w}{i}") for i in range(4)]
        kT = [pin.tile([128, KLE], F32R, tag=f"kTi{i}", name=f"kTi{i}") for i in range(4)]
        qT = [pin.tile([128, QS], F32R, tag=f"qTi{i}", name=f"qTi{i}") for i in range(4)]
        vT = [pin.tile([128, KLE], F32R, tag=f"vTi{i}", name=f"vTi{i}") for i in range(4)]
        # load order ~ consumption order
        for dk in range(4):
            nc.sync.dma_start(wsb["wkT"][dk][:], w_d["wkT"].rearrange("(t p) d -> t p d", p=128)[dk])
        for dk in range(4):
            nc.sync.dma_start(kT[dk][:], kT_d.rearrange("(t p) d -> t p d", p=128)[dk])
        for dk in range(4):
            nc.sync.dma_start(wsb["wqT"][dk][:], w_d["wqT"].rearrange("(t p) d -> t p d", p=128)[dk])
        for dk in range(4):
            nc.sync.dma_start(qT[dk][:], qT_d.rearrange("(t p) d -> t p d", p=128)[dk])
        for dk in range(4):
            nc.sync.dma_start(wsb["wvT"][dk][:], w_d["wvT"].rearrange("(t p) d -> t p d", p=128)[dk])
        for dk in range(4):
            nc.sync.dma_start(vT[dk][:], vT_d.rearrange("(t p) d -> t p d", p=128)[dk])
        for dk in range(4):
            nc.sync.dma_start(wsb["woT"][dk][:], w_d["woT"].rearrange("(t p) d -> t p d", p=128)[dk])

        # ---- projected tensors ------------------------------------------
        pp = ctx.enter_context(tc.tile_pool(name="proj", bufs=1))
        KT = [pp.tile([128, KLE], F32R, tag=f"KT{i}", name=f"KT{i}") for i in range(4)]
        QT = [pp.tile([128, QS], F32R, tag=f"QT{i}", name=f"QT{i}") for i in range(4)]
        VS = [pp.tile([128, H * (HD + 1)], BF16, tag=f"VS{i}", name=f"VS{i}") for i in range(NJT)]

        # ---- phase P: projections (no transposes) -----------------------
        with tc.tile_pool(name="psumP", bufs=4, space="PSUM") as ppsP:
            nevac = 0

            def evac(dst, src):
                # alternate evacuation engine; ACT is otherwise idle here
                nonlocal nevac
                if nevac % 2 == 0:
                    nc.vector.tensor_copy(dst, src)
                else:
                    nc.scalar.copy(dst, src)
                nevac += 1

            # K projection: KT[ot][do, j] = sum_d wkT[d, ot*128+do] * kT[d, j]
            for ot in range(4):
                for j0 in range(0, KLE, 512):
                    jw = min(512, KLE - j0)
                    ps = ppsP.tile([128, 512], F32, tag="pp", name=f"ps_k{ot}_{j0}")
                    for dk in range(4):
                        nc.tensor.matmul(
                            ps[:, 0:jw],
                            wsb["wkT"][dk][:, ot * 128:(ot + 1) * 128],
                            kT[dk][:, j0:j0 + jw],
                            start=(dk == 0), stop=(dk == 3),
                        )
                    evac(KT[ot][:, j0:j0 + jw], ps[:, 0:jw])
            # Q projection
            for ot in range(4):
                for ic in range(QS // 512):
                    ps = ppsP.tile([128, 512], F32, tag="pp", name=f"ps_q{ot}_{ic}")
                    for dk in range(4):
                        nc.tensor.matmul(
                            ps[:],
                            wsb["wqT"][dk][:, ot * 128:(ot + 1) * 128],
                            qT[dk][:, ic * 512:(ic + 1) * 512],
                            start=(dk == 0), stop=(dk == 3),
                        )
                    evac(QT[ot][:, ic * 512:(ic + 1) * 512], ps[:])
            # V projection: VS[jt][j, :] = head_v rows * mask, + mask column
            for jt in range(NJT):
                ps = ppsP.tile([128, 512], F32, tag="pp", name=f"ps_v{jt}")
                for dk in range(4):
                    nc.tensor.matmul(
                        ps[:],
                        vT[dk][:, jt * 128:(jt + 1) * 128],
                        wsb["wvT"][dk][:],
                        start=(dk == 0), stop=(dk == 3),
                    )
                vs_out = VS[jt][:].rearrange("p (h d) -> p h d", d=HD + 1)
                nc.vector.tensor_scalar(
                    vs_out[:, :, 0:HD],
                    ps[:].rearrange("p (h d) -> p h d", d=HD),
                    m_sb[:, jt:jt + 1],
                    None,
                    mybir.AluOpType.mult,
                )
                nc.vector.tensor_copy(
                    vs_out[:, :, HD].squeeze(),
                    m_sb[:, jt:jt + 1].broadcast_to([128, H]),
                )

        # ---- phase D: attention -----------------------------------------
        # Head pairs share PSUM partition halves: even head at 0..63, odd at
        # 64..127 in the score tile -> one wide exp covers both.  A2 packs
        # the normalized pair the same way, feeding a K=128 Wo contraction.
        pA = ctx.enter_context(tc.tile_pool(name="attn_out", bufs=1))
        A2 = [pA.tile([128, QS], F32R, tag=f"A2{hp}", name=f"A2{hp}") for hp in range(4)]
        ones = pA.tile([128, HD], F32R, tag="ones_r")
        nc.vector.tensor_copy(ones[:], _r(ones_f[:]))
        with tc.tile_pool(name="eP", bufs=6) as pe, \
             tc.tile_pool(name="rP", bufs=2) as pr, \
             tc.tile_pool(name="bP", bufs=2) as pb, \
             tc.tile_pool(name="psumD", bufs=1, space="PSUM") as ppsD:
            for ic in range(QS // 512):
                i0 = ic * 512
                for hp in range(H // 2):
                    he, ho = 2 * hp, 2 * hp + 1
                    pv_e = ppsD.tile([65, 512], F32, tag="pvbc", bufs=4, name=f"pve{hp}_{ic}")
                    pv_o = ppsD.tile([65, 512], F32, tag="pvbc", bufs=4, name=f"pvo{hp}_{ic}")
                    for jt in range(NJT):
                        s_ps = ppsD.tile([128, 1024], F32, tag="s", bufs=2, name=f"s{hp}_{ic}_{jt}")
                        for po2, sl in ((0, slice(0, 512)), (HD, slice(512, 1024))):
                            nc.tensor.matmul(
                                s_ps[:, sl],
                                KT[hp][po2:po2 + HD, jt * 128:(jt + 1) * 128],
                                QT[hp][po2:po2 + HD, i0:i0 + 512],
                                start=True, stop=True,
                            )
                        e_t = pe.tile([128, 1024], BF16, tag="e", name=f"e{hp}_{ic}_{jt}")
                        nc.scalar.activation(e_t[:], s_ps[:], EXP, scale=SCALE, bias=ebias[:, 0:1])
                        nc.tensor.matmul(
                            pv_e[0:65, :],
                            VS[jt][:, he * (HD + 1):(he + 1) * (HD + 1)],
                            e_t[:, 0:512],
                            start=(jt == 0), stop=(jt == NJT - 1),
                        )
                        nc.tensor.matmul(
                            pv_o[0:65, :],
                            VS[jt][:, ho * (HD + 1):(ho + 1) * (HD + 1)],
                            e_t[:, 512:1024],
                            start=(jt == 0), stop=(jt == NJT - 1),
                        )
                    for po, pv in ((0, pv_e), (HD, pv_o)):
                        h = 2 * hp + (po != 0)
                        r_sb = pr.tile([65, 512], F32R, tag="r", name=f"r{h}_{ic}")
                        ln_t = pr.tile([65, 512], F32, tag="ln", name=f"ln{h}_{ic}")
                        nc.scalar.activation(ln_t[64:65, :], pv[64:65, :], LN)
                        nc.scalar.activation(r_sb[64:65, :], ln_t[64:65, :], EXP, scale=-1.0)
                        bc = ppsD.tile([HD, 512], F32, tag="pvbc", bufs=4, name=f"bc{h}_{ic}")
                        nc.tensor.matmul(
                            bc[:, :],
                            ones[64:65, 0:HD],
                            r_sb[64:65, :],
                            start=True, stop=True,
                        )
                        bc_sb = pb.tile([HD, 512], F32, tag="bcs", name=f"bcs{h}_{ic}")
                        nc.vector.tensor_copy(bc_sb[:], bc[:])
                        nc.vector.tensor_mul(
                            A2[hp][po:po + HD, i0:i0 + 512], pv[0:HD, :], bc_sb[:]
                        )
                # output projection for this query chunk (overlaps next ic)
                for itl in range(4):
                    c0 = i0 + itl * 128
                    o_ps = ppsD.tile([128, D], F32, tag="pvbc", bufs=4, name=f"ops{ic}_{itl}")
                    for hp in range(4):
                        nc.tensor.matmul(
                            o_ps[:],
                            A2[hp][:, c0:c0 + 128],
                            wsb["woT"][hp][:],
                            start=(hp == 0),
                            stop=(hp == 3),
                        )
                    o_sb = pb.tile([128, D], F32, tag="osb", name=f"osb{ic}_{itl}")
                    nc.vector.tensor_copy(o_sb[:], o_ps[:])
                    nc.sync.dma_start(out_d[c0:c0 + 128, :], o_sb[:])

    return nc


_NC_CACHE = {}


def _get_nc(KLE):
    if KLE not in _NC_CACHE:
        nc = build_kernel(KLE)
        _legalize_waits(nc)
        _NC_CACHE[KLE] = nc
    return _NC_CACHE[KLE]


def shard_inputs(query, key, value, Wq, Wk, Wv, Wo, attn_mask):
    """Per-core shards.  Masked kv rows are dropped (order-invariant under
    softmax; fully-masked rows contribute exactly 0), survivors packed into
    a KLE-row buffer (KLE = max count over batches, rounded up to 128),
    and everything is shipped pre-transposed."""
    idxs = [np.nonzero(np.asarray(attn_mask[b]) != 0)[0] for b in range(B)]
    maxcnt = max((len(ix) for ix in idxs), default=1)
    KLE = max(128, -(-maxcnt // 128) * 128)
    wqT = np.ascontiguousarray(np.asarray(Wq, np.float32).T)
    wkT = np.ascontiguousarray(np.asarray(Wk, np.float32).T)
    wvT = np.ascontiguousarray(np.asarray(Wv, np.float32).T)
    woT = np.ascontiguousarray(np.asarray(Wo, np.float32).T)
    in_maps = []
    for c in range(8):
        b, half = c // 2, c % 2
        idx = idxs[b]
        kc = np.zeros((D, KLE), np.float32)
        vc = np.zeros((D, KLE), np.float32)
        kc[:, : len(idx)] = np.asarray(key[b], np.float32)[idx].T
        vc[:, : len(idx)] = np.asarray(value[b], np.float32)[idx].T
        mf = np.zeros(KLE, np.float32)
        mf[: len(idx)] = 1.0
        in_maps.append({
            "qT": np.ascontiguousarray(
                np.asarray(query[b, half * QS:(half + 1) * QS], np.float32).T
            ),
            "kT": kc,
            "vT": vc,
            "wqT": wqT, "wkT": wkT, "wvT": wvT, "woT": woT,
            "mask2d": np.ascontiguousarray(mf.reshape(KLE // 128, 128).T),
        })
    return in_maps, KLE


def kernel(query, key, value, Wq, Wk, Wv, Wo, attn_mask, _trace=False, _trace_kwargs=None):
    from concourse.bass_utils import run_bass_kernel_spmd

    in_maps, KLE = shard_inputs(query, key, value, Wq, Wk, Wv, Wo, attn_mask)
    nc = _get_nc(KLE)
    res = run_bass_kernel_spmd(
        nc, in_maps, list(range(8)), trace=_trace, **(_trace_kwargs or {})
    )
    out = np.empty((B, Q, D), dtype=np.float32)
    for c in range(8):
        b, half = c // 2, c % 2
        out[b, half * QS:(half + 1) * QS] = res.results[c]["out"]
    if _trace:
        kernel._last_results = res
    return out


# revision 5
# speedup vs baseline: 1.1479x; 1.1479x over previous
"""Multi-head attention Bass/Tile kernel for Trainium2, 8-core SPMD — v3.

v3 over v2: phase D is ACT(exp)-bound, so the PE queue is software-pipelined
(next score tile issues before the current PV pair), the per-head Ln/Exp
normalization is batched into one [8,512] Ln + Exp per query chunk (den rows
gathered by tiny PSUM->SBUF DMAs, reciprocal broadcast via a one-hot selector
matmul), and the K/Q projections for later head-pairs plus the ic0 output
projection are interleaved into the attention loops to fill PE slack.
"""

import sys

if "/opt/trn_rl_repo" not in sys.path:
    sys.path.insert(0, "/opt/trn_rl_repo")

from contextlib import ExitStack

import numpy as np

import concourse.bass as bass
import concourse.tile as tile
from concourse import mybir
import bass_rust as _bass_rust

F32 = mybir.dt.float32
F32R = mybir.dt.float32r
BF16 = mybir.dt.bfloat16
EXP = mybir.ActivationFunctionType.Exp
LN = mybir.ActivationFunctionType.Ln

B, Q, KL, D, H = 4, 2048, 2048, 512, 8
HD = D // H            # 64
QS = Q // 2            # 1024 query rows per core
SCALE = 1.0 / HD ** 0.5
EXPBIAS = -30.0


def _legalize_waits(nc, max_waits=1):
    n = 0
    for f in nc.m.functions:
        for bb in f.blocks:
            insts = bb.instructions
            i = 0
            while i < len(insts):
                inst = insts[i]
                si = inst.sync_info
                if si is not None and len(si.on_wait) > max_waits:
                    waits = list(si.on_wait)
                    for j, w in enumerate(waits[max_waits:]):
                        nop = mybir.InstNoOp(
                            name=f"{inst.name}-waitsplit{j}", ins=[], outs=[]
                        )
                        nop.engine = inst.engine
                        nop.sync_info = _bass_rust.SyncInfo(on_wait=[w], on_update=[])
                        insts.insert(i, nop)
                        i += 1
                        n += 1
                    inst.sync_info = _bass_rust.SyncInfo(
                        on_wait=waits[:max_waits], on_update=list(si.on_update)
                    )
                i += 1
    return n


def build_kernel(KLE):
    NJT = KLE // 128
    nc = bass.Bass("TRN2", target_bir_lowering=False, debug=False)

    qT_d = nc.dram_tensor("qT", [D, QS], F32R, kind="ExternalInput").ap()
    kT_d = nc.dram_tensor("kT", [D, KLE], F32R, kind="ExternalInput").ap()
    vT_d = nc.dram_tensor("vT", [D, KLE], F32R, kind="ExternalInput").ap()
    w_d = {
        w: nc.dram_tensor(w, [D, D], F32R, kind="ExternalInput").ap()
        for w in ("wqT", "wkT", "wvT", "woT")
    }
    m_d = nc.dram_tensor("mask2d", [128, NJT], F32, kind="ExternalInput").ap()
    out_d = nc.dram_tensor("out", [QS, D], F32, kind="ExternalOutput").ap()

    # one-hot selector: sel[p, hp*128 + m] = 1 if p == 2*hp + (m >= 64)
    sel_np = np.zeros((8, 512), np.float32)
    for hp in range(4):
        sel_np[2 * hp, hp * 128:hp * 128 + 64] = 1.0
        sel_np[2 * hp + 1, hp * 128 + 64:hp * 128 + 128] = 1.0
    sel_d = nc.inline_tensor(sel_np, name="sel")

    with tile.TileContext(nc) as tc, ExitStack() as ctx:
        pc = ctx.enter_context(tc.tile_pool(name="const", bufs=1))
        m_sb = pc.tile([128, NJT], F32, tag="m_sb")
        nc.sync.dma_start(m_sb[:], m_d)
        ebias = pc.tile([128, 1], F32, tag="ebias")
        nc.vector.memset(ebias[:], EXPBIAS)
        sel_f = pc.tile([8, 512], F32, tag="sel_f")
        nc.sync.dma_start(sel_f[:], sel_d.ap())
        sel = pc.tile([8, 512], F32R, tag="sel")
        nc.vector.tensor_copy(sel[:], sel_f[:])

        # ---- input tiles (DMA direct, pre-transposed on host) -----------
        pin = ctx.enter_context(tc.tile_pool(name="inputs", bufs=1))
        wsb = {}
        for w in ("wkT", "wqT", "wvT", "woT"):
            wsb[w] = [pin.tile([128, D], F32R, tag=f"{w}{i}", name=f"{w}{i}") for i in range(4)]
        kT = [pin.tile([128, KLE], F32R, tag=f"kTi{i}", name=f"kTi{i}") for i in range(4)]
        qT = [pin.tile([128, QS], F32R, tag=f"qTi{i}", name=f"qTi{i}") for i in range(4)]
        vT = [pin.tile([128, KLE], F32R, tag=f"vTi{i}", name=f"vTi{i}") for i in range(4)]
        for dk in range(4):
            nc.sync.dma_start(wsb["wkT"][dk][:], w_d["wkT"].rearrange("(t p) d -> t p d", p=128)[dk])
        for dk in range(4):
            nc.sync.dma_start(kT[dk][:], kT_d.rearrange("(t p) d -> t p d", p=128)[dk])
        for dk in range(4):
            nc.sync.dma_start(wsb["wqT"][dk][:], w_d["wqT"].rearrange("(t p) d -> t p d", p=128)[dk])
        for dk in range(4):
            nc.sync.dma_start(qT[dk][:], qT_d.rearrange("(t p) d -> t p d", p=128)[dk])
        for dk in range(4):
            nc.sync.dma_start(wsb["wvT"][dk][:], w_d["wvT"].rearrange("(t p) d -> t p d", p=128)[dk])
        for dk in range(4):
            nc.sync.dma_start(vT[dk][:], vT_d.rearrange("(t p) d -> t p d", p=128)[dk])
        for dk in range(4):
            nc.sync.dma_start(wsb["woT"][dk][:], w_d["woT"].rearrange("(t p) d -> t p d", p=128)[dk])

        pp = ctx.enter_context(tc.tile_pool(name="proj", bufs=1))
        KT = [pp.tile([128, KLE], F32R, tag=f"KT{i}", name=f"KT{i}") for i in range(4)]
        QT = [pp.tile([128, QS], F32R, tag=f"QT{i}", name=f"QT{i}") for i in range(4)]
        VS = [pp.tile([128, H * (HD + 1)], BF16, tag=f"VS{i}", name=f"VS{i}") for i in range(NJT)]
        pA = ctx.enter_context(tc.tile_pool(name="attn_out", bufs=1))
        A2r = [pA.tile([128, QS], F32, tag=f"A2r{hp}", name=f"A2r{hp}") for hp in range(4)]
        A2 = [pA.tile([128, QS], F32R, tag=f"A2{hp}", name=f"A2{hp}") for hp in range(4)]

        with tc.tile_pool(name="eP", bufs=6) as pe_pool, \
             tc.tile_pool(name="rP", bufs=2) as pr, \
             tc.tile_pool(name="bP", bufs=2) as pb, \
             tc.tile_pool(name="psumS", bufs=2, space="PSUM") as ppsS, \
             tc.tile_pool(name="psumPV", bufs=2, space="PSUM") as ppsPV, \
             tc.tile_pool(name="psumX", bufs=2, space="PSUM") as ppsX:

            # ---- deferred work-item machinery (PE filler groups) --------
            nevac = [0]

            def evac(dst, src, eng="v"):
                if eng == "v":
                    nc.vector.tensor_copy(dst, src)
                else:
                    nc.scalar.copy(dst, src)

            def k_group(ot, j0, eng):
                jw = min(512, KLE - j0)
                ps = ppsX.tile([128, 512], F32, tag="aux", name=f"ps_k{ot}_{j0}")
                for dk in range(4):
                    nc.tensor.matmul(
                        ps[:, 0:jw],
                        wsb["wkT"][dk][:, ot * 128:(ot + 1) * 128],
                        kT[dk][:, j0:j0 + jw],
                        start=(dk == 0), stop=(dk == 3),
                    )
                evac(KT[ot][:, j0:j0 + jw], ps[:, 0:jw], eng)

            def q_group(ot, icc, eng):
                ps = ppsX.tile([128, 512], F32, tag="aux", name=f"ps_q{ot}_{icc}")
                for dk in range(4):
                    nc.tensor.matmul(
                        ps[:],
                        wsb["wqT"][dk][:, ot * 128:(ot + 1) * 128],
                        qT[dk][:, icc * 512:(icc + 1) * 512],
                        start=(dk == 0), stop=(dk == 3),
                    )
                evac(QT[ot][:, icc * 512:(icc + 1) * 512], ps[:], eng)

            def v_group(jt, eng):
                ps = ppsX.tile([128, 512], F32, tag="aux", name=f"ps_v{jt}")
                for dk in range(4):
                    nc.tensor.matmul(
                        ps[:],
                        vT[dk][:, jt * 128:(jt + 1) * 128],
                        wsb["wvT"][dk][:],
                        start=(dk == 0), stop=(dk == 3),
                    )
                vs_out = VS[jt][:].rearrange("p (h d) -> p h d", d=HD + 1)
                nc.vector.tensor_scalar(
                    vs_out[:, :, 0:HD],
                    ps[:].rearrange("p (h d) -> p h d", d=HD),
                    m_sb[:, jt:jt + 1],
                    None,
                    mybir.AluOpType.mult,
                )
                nc.vector.tensor_copy(
                    vs_out[:, :, HD].squeeze(),
                    m_sb[:, jt:jt + 1].broadcast_to([128, H]),
                )

            def wo_group(itl, ic):
                c0 = ic * 512 + itl * 128
                o_ps = ppsX.tile([128, D], F32, tag="aux", name=f"ops{ic}_{itl}")
                for hp in range(4):
                    nc.tensor.matmul(
                        o_ps[:],
                        A2[hp][:, c0:c0 + 128],
                        wsb["woT"][hp][:],
                        start=(hp == 0),
                        stop=(hp == 3),
                    )
                o_sb = pb.tile([128, D], F32, tag="osb", name=f"osb{ic}_{itl}")
                nc.vector.tensor_copy(o_sb[:], o_ps[:])
                nc.sync.dma_start(out_d[c0:c0 + 128, :], o_sb[:])

            # ---- prefix: K0, Q0, V(all) — ACT idle anyway ----------------
            k_group(0, 0, "s")
            k_group(0, 512, "v")
            if KLE > 1024:
                k_group(0, 1024, "s")
            q_group(0, 0, "v")
            q_group(0, 1, "s")
            for jt in range(NJT):
                v_group(jt, "v")

            # filler queues: ic0 gets remaining K/Q projections, ic1 gets
            # the ic0 output projection.  One group ~= one PE slack slot.
            filler = {0: [], 1: []}
            for ot in range(1, 4):
                for j0 in range(0, KLE, 512):
                    filler[0].append(("k", ot, j0))
                for icc in range(2):
                    filler[0].append(("q", ot, icc))
            for itl in range(4):
                filler[1].append(("wo", itl, 0))

            def run_filler(ic, hp, jt):
                # during hp's loop only groups for ot <= hp+1 are safe
                # (ot = hp+1 must finish before hp+1's first score)
                if not filler[ic]:
                    return
                kind = filler[ic][0][0]
                if kind in ("k", "q") and filler[ic][0][1] > hp + 1:
                    return
                item = filler[ic].pop(0)
                if item[0] == "k":
                    k_group(item[1], item[2], "v")
                elif item[0] == "q":
                    q_group(item[1], item[2], "v")
                else:
                    wo_group(item[1], item[2])

            # ---- attention ----------------------------------------------
            for ic in range(2):
                i0 = ic * 512
                den_g = pr.tile([8, 512], F32, tag="deng", name=f"deng{ic}")
                pv_tiles = []
                for hp in range(4):
                    he, ho = 2 * hp, 2 * hp + 1
                    pv_e = ppsPV.tile([65, 512], F32, tag="pv", name=f"pve{hp}_{ic}")
                    pv_o = ppsPV.tile([65, 512], F32, tag="pv", name=f"pvo{hp}_{ic}")

                    def s_mm(jt):
                        s_ps = ppsS.tile([128, 1024], F32, tag="s", name=f"s{hp}_{ic}_{jt}")
                        for po2, sl in ((0, slice(0, 512)), (HD, slice(512, 1024))):
                            nc.tensor.matmul(
                                s_ps[:, sl],
                                KT[hp][po2:po2 + HD, jt * 128:(jt + 1) * 128],
                                QT[hp][po2:po2 + HD, i0:i0 + 512],
                                start=True, stop=True,
                            )
                        return s_ps

                    s_cur = s_mm(0)
                    for jt in range(NJT):
                        e_t = pe_pool.tile([128, 1024], BF16, tag="e", name=f"e{hp}_{ic}_{jt}")
                        nc.scalar.activation(e_t[:], s_cur[:], EXP, scale=SCALE, bias=ebias[:, 0:1])
                        if jt + 1 < NJT:
                            s_cur = s_mm(jt + 1)
                        nc.tensor.matmul(
                            pv_e[0:65, :],
                            VS[jt][:, he * (HD + 1):(he + 1) * (HD + 1)],
                            e_t[:, 0:512],
                            start=(jt == 0), stop=(jt == NJT - 1),
                        )
                        nc.tensor.matmul(
                            pv_o[0:65, :],
                            VS[jt][:, ho * (HD + 1):(ho + 1) * (HD + 1)],
                            e_t[:, 512:1024],
                            start=(jt == 0), stop=(jt == NJT - 1),
                        )
                        if jt % 2 == 1:
                            run_filler(ic, hp, jt)
                    # evacuate raw pair + gather denominators; frees pv slots
                    nc.vector.tensor_copy(A2r[hp][0:HD, i0:i0 + 512], pv_e[0:HD, :])
                    nc.vector.tensor_copy(A2r[hp][HD:128, i0:i0 + 512], pv_o[0:HD, :])
                    # DMA can't read PSUM: bounce den rows through partition 64
                    # of an SBUF stage, then cross-partition SBUF->SBUF DMA
                    stage = pb.tile([65, 1024], F32, tag="dstage", name=f"dst{hp}_{ic}")
                    nc.vector.tensor_copy(stage[64:65, 0:512], pv_e[64:65, :])
                    nc.vector.tensor_copy(stage[64:65, 512:1024], pv_o[64:65, :])
                    nc.sync.dma_start(den_g[he:he + 1, :], stage[64:65, 0:512])
                    nc.sync.dma_start(den_g[ho:ho + 1, :], stage[64:65, 512:1024])
                    run_filler(ic, hp, -1)

                # batched normalization for this query chunk
                ln_g = pr.tile([8, 512], F32, tag="lng", name=f"lng{ic}")
                r_g = pr.tile([8, 512], F32R, tag="rg", name=f"rg{ic}")
                nc.scalar.activation(ln_g[:], den_g[:], LN)
                nc.scalar.activation(r_g[:], ln_g[:], EXP, scale=-1.0)
                for hp in range(4):
                    bc = ppsX.tile([128, 512], F32, tag="aux", name=f"bc{hp}_{ic}")
                    nc.tensor.matmul(
                        bc[:],
                        sel[:, hp * 128:(hp + 1) * 128],
                        r_g[:],
                        start=True, stop=True,
                    )
                    nc.vector.tensor_mul(
                        A2[hp][:, i0:i0 + 512], A2r[hp][:, i0:i0 + 512], bc[:]
                    )
            # tail: second-chunk output projection
            for itl in range(4):
                wo_group(itl, 1)

    return nc


_NC_CACHE = {}


def _get_nc(KLE):
    if KLE not in _NC_CACHE:
        nc = build_kernel(KLE)
        _legalize_waits(nc)
        _NC_CACHE[KLE] = nc
    return _NC_CACHE[KLE]


def shard_inputs(query, key, value, Wq, Wk, Wv, Wo, attn_mask):
    idxs = [np.nonzero(np.asarray(attn_mask[b]) != 0)[0] for b in range(B)]
    maxcnt = max((len(ix) for ix in idxs), default=1)
    KLE = max(128, -(-maxcnt // 128) * 128)
    wqT = np.ascontiguousarray(np.asarray(Wq, np.float32).T)
    wkT = np.ascontiguousarray(np.asarray(Wk, np.float32).T)
    wvT = np.ascontiguousarray(np.asarray(Wv, np.float32).T)
    woT = np.ascontiguousarray(np.asarray(Wo, np.float32).T)
    in_maps = []
    for c in range(8):
        b, half = c // 2, c % 2
        idx = idxs[b]
        kc = np.zeros((D, KLE), np.float32)
        vc = np.zeros((D, KLE), np.float32)
        kc[:, : len(idx)] = np.asarray(key[b], np.float32)[idx].T
        vc[:, : len(idx)] = np.asarray(value[b], np.float32)[idx].T
        mf = np.zeros(KLE, np.float32)
        mf[: len(idx)] = 1.0
        in_maps.append({
            "qT": np.ascontiguousarray(
                np.asarray(query[b, half * QS:(half + 1) * QS], np.float32).T
            ),
            "kT": kc,
            "vT": vc,
            "wqT": wqT, "wkT": wkT, "wvT": wvT, "woT": woT,
            "mask2d": np.ascontiguousarray(mf.reshape(KLE // 128, 128).T),
        })
    return in_maps, KLE


def kernel(query, key, value, Wq, Wk, Wv, Wo, attn_mask, _trace=False, _trace_kwargs=None):
    from concourse.bass_utils import run_bass_kernel_spmd

    in_maps, KLE = shard_inputs(query, key, value, Wq, Wk, Wv, Wo, attn_mask)
    nc = _get_nc(KLE)
    res = run_bass_kernel_spmd(
        nc, in_maps, list(range(8)), trace=_trace, **(_trace_kwargs or {})
    )
    out = np.empty((B, Q, D), dtype=np.float32)
    for c in range(8):
        b, half = c // 2, c % 2
        out[b, half * QS:(half + 1) * QS] = res.results[c]["out"]
    if _trace:
        kernel._last_results = res
    return out


# revision 7
# speedup vs baseline: 1.2360x; 1.0767x over previous
"""Multi-head attention Bass/Tile kernel for Trainium2, 8-core SPMD — v3.

v3 over v2: phase D is ACT(exp)-bound, so the PE queue is software-pipelined
(next score tile issues before the current PV pair), the per-head Ln/Exp
normalization is batched into one [8,512] Ln + Exp per query chunk (den rows
gathered by tiny PSUM->SBUF DMAs, reciprocal broadcast via a one-hot selector
matmul), and the K/Q projections for later head-pairs plus the ic0 output
projection are interleaved into the attention loops to fill PE slack.
"""

import sys

if "/opt/trn_rl_repo" not in sys.path:
    sys.path.insert(0, "/opt/trn_rl_repo")

from contextlib import ExitStack

import numpy as np
import ml_dtypes

_BF16NP = ml_dtypes.bfloat16

import concourse.bass as bass
import concourse.tile as tile
from concourse import mybir
import bass_rust as _bass_rust

F32 = mybir.dt.float32
F32R = mybir.dt.float32r
BF16 = mybir.dt.bfloat16
EXP = mybir.ActivationFunctionType.Exp
LN = mybir.ActivationFunctionType.Ln

B, Q, KL, D, H = 4, 2048, 2048, 512, 8
HD = D // H            # 64
QS = Q // 2            # 1024 query rows per core
SCALE = 1.0 / HD ** 0.5
EXPBIAS = -30.0


def _legalize_waits(nc, max_waits=1):
    n = 0
    for f in nc.m.functions:
        for bb in f.blocks:
            insts = bb.instructions
            i = 0
            while i < len(insts):
                inst = insts[i]
                si = inst.sync_info
                if si is not None and len(si.on_wait) > max_waits:
                    waits = list(si.on_wait)
                    for j, w in enumerate(waits[max_waits:]):
                        nop = mybir.InstNoOp(
                            name=f"{inst.name}-waitsplit{j}", ins=[], outs=[]
                        )
                        nop.engine = inst.engine
                        nop.sync_info = _bass_rust.SyncInfo(on_wait=[w], on_update=[])
                        insts.insert(i, nop)
                        i += 1
                        n += 1
                    inst.sync_info = _bass_rust.SyncInfo(
                        on_wait=waits[:max_waits], on_update=list(si.on_update)
                    )
                i += 1
    return n


def build_kernel(KLE):
    NJT = KLE // 128
    nc = bass.Bass("TRN2", target_bir_lowering=False, debug=False)

    qT_d = nc.dram_tensor("qT", [D, QS], F32R, kind="ExternalInput").ap()
    kT_d = nc.dram_tensor("kT", [D, KLE], F32R, kind="ExternalInput").ap()
    vT_d = nc.dram_tensor("vT", [D, KLE], BF16, kind="ExternalInput").ap()
    w_d = {
        w: nc.dram_tensor(w, [D, D], F32R, kind="ExternalInput").ap()
        for w in ("wqT", "wkT", "woT")
    }
    w_d["wvT"] = nc.dram_tensor("wvT", [D, D], BF16, kind="ExternalInput").ap()
    m_d = nc.dram_tensor("mask2d", [128, NJT], F32, kind="ExternalInput").ap()
    out_d = nc.dram_tensor("out", [QS, D], F32, kind="ExternalOutput").ap()

    # one-hot selector: sel[p, hp*128 + m] = 1 if p == 2*hp + (m >= 64)
    sel_np = np.zeros((8, 512), np.float32)
    for hp in range(4):
        sel_np[2 * hp, hp * 128:hp * 128 + 64] = 1.0
        sel_np[2 * hp + 1, hp * 128 + 64:hp * 128 + 128] = 1.0
    sel_d = nc.inline_tensor(sel_np, name="sel")

    with tile.TileContext(nc) as tc, ExitStack() as ctx:
        pc = ctx.enter_context(tc.tile_pool(name="const", bufs=1))
        m_sb = pc.tile([128, NJT], F32, tag="m_sb")
        nc.sync.dma_start(m_sb[:], m_d)
        ebias = pc.tile([128, 1], F32, tag="ebias")
        nc.vector.memset(ebias[:], EXPBIAS)
        sel_f = pc.tile([8, 512], F32, tag="sel_f")
        nc.sync.dma_start(sel_f[:], sel_d.ap())
        sel = pc.tile([8, 512], F32R, tag="sel")
        nc.vector.tensor_copy(sel[:], sel_f[:])

        # ---- input tiles (DMA direct, pre-transposed on host) -----------
        pin = ctx.enter_context(tc.tile_pool(name="inputs", bufs=1))
        wsb = {}
        for w in ("wkT", "wqT", "woT"):
            wsb[w] = [pin.tile([128, D], F32R, tag=f"{w}{i}", name=f"{w}{i}") for i in range(4)]
        wsb["wvT"] = [pin.tile([128, D], BF16, tag=f"wvT{i}", name=f"wvT{i}") for i in range(4)]
        kT = [pin.tile([128, KLE], F32R, tag=f"kTi{i}", name=f"kTi{i}") for i in range(4)]
        qT = [pin.tile([128, QS], F32R, tag=f"qTi{i}", name=f"qTi{i}") for i in range(4)]
        vT = [pin.tile([128, KLE], BF16, tag=f"vTi{i}", name=f"vTi{i}") for i in range(4)]
        # DMA emission order ~ earliest consumer: scores need wq/q-half/wk/k,
        # then the V path streams in per column chunk just ahead of its PV use
        for dk in range(4):
            nc.sync.dma_start(wsb["wqT"][dk][:], w_d["wqT"].rearrange("(t p) d -> t p d", p=128)[dk])
        for dk in range(4):
            nc.sync.dma_start(qT[dk][:, 0:512], qT_d.rearrange("(t p) d -> t p d", p=128)[dk][:, 0:512])
        for dk in range(4):
            nc.sync.dma_start(wsb["wkT"][dk][:], w_d["wkT"].rearrange("(t p) d -> t p d", p=128)[dk])
        for c0 in range(0, KLE, 512):
            cw = min(512, KLE - c0)
            for dk in range(4):
                nc.sync.dma_start(
                    kT[dk][:, c0:c0 + cw],
                    kT_d.rearrange("(t p) d -> t p d", p=128)[dk][:, c0:c0 + cw],
                )
        for dk in range(4):
            nc.sync.dma_start(wsb["wvT"][dk][:], w_d["wvT"].rearrange("(t p) d -> t p d", p=128)[dk])
        for c0 in range(0, KLE, 384):
            cw = min(384, KLE - c0)
            for dk in range(4):
                nc.sync.dma_start(
                    vT[dk][:, c0:c0 + cw],
                    vT_d.rearrange("(t p) d -> t p d", p=128)[dk][:, c0:c0 + cw],
                )
        for dk in range(4):
            nc.sync.dma_start(qT[dk][:, 512:1024], qT_d.rearrange("(t p) d -> t p d", p=128)[dk][:, 512:1024])
        for dk in range(4):
            nc.sync.dma_start(wsb["woT"][dk][:], w_d["woT"].rearrange("(t p) d -> t p d", p=128)[dk])

        pp = ctx.enter_context(tc.tile_pool(name="proj", bufs=1))
        KT = [pp.tile([128, KLE], F32R, tag=f"KT{i}", name=f"KT{i}") for i in range(4)]
        QT = [pp.tile([128, QS], F32R, tag=f"QT{i}", name=f"QT{i}") for i in range(4)]
        VS = [pp.tile([128, H * (HD + 1)], BF16, tag=f"VS{i}", name=f"VS{i}") for i in range(NJT)]
        pA = ctx.enter_context(tc.tile_pool(name="attn_out", bufs=1))
        A2r = [pA.tile([128, QS], F32, tag=f"A2r{hp}", name=f"A2r{hp}") for hp in range(4)]
        A2 = [pA.tile([128, QS], F32R, tag=f"A2{hp}", name=f"A2{hp}") for hp in range(4)]

        with tc.tile_pool(name="eP", bufs=6) as pe_pool, \
             tc.tile_pool(name="rP", bufs=2) as pr, \
             tc.tile_pool(name="bP", bufs=2) as pb, \
             tc.tile_pool(name="psumS", bufs=2, space="PSUM") as ppsS, \
             tc.tile_pool(name="psumPV", bufs=2, space="PSUM") as ppsPV, \
             tc.tile_pool(name="psumX", bufs=2, space="PSUM") as ppsX:

            # ---- deferred work-item machinery (PE filler groups) --------
            nevac = [0]

            def evac(dst, src, eng="v"):
                if eng == "v":
                    nc.vector.tensor_copy(dst, src)
                else:
                    nc.scalar.copy(dst, src)

            # accumulation chains interleaved across two PSUM tiles so one
            # chain's drain overlaps the other's stream
            def _mm_pair(specs):
                tiles = [
                    ppsX.tile([128, 512], F32, tag="aux", name=f"ps_{nm}")
                    for nm, _, _, _ in specs
                ]
                for dk in range(4):
                    for t, (nm, w, lhs_fn, rhs_fn) in zip(tiles, specs):
                        nc.tensor.matmul(
                            t[:, 0:w],
                            lhs_fn(dk), rhs_fn(dk),
                            start=(dk == 0), stop=(dk == 3),
                        )
                return tiles

            def k_spec(ot, j0):
                jw = min(512, KLE - j0)
                return (f"k{ot}_{j0}", jw,
                        lambda dk: wsb["wkT"][dk][:, ot * 128:(ot + 1) * 128],
                        lambda dk: kT[dk][:, j0:j0 + jw])

            def q_spec(ot, icc):
                return (f"q{ot}_{icc}", 512,
                        lambda dk: wsb["wqT"][dk][:, ot * 128:(ot + 1) * 128],
                        lambda dk: qT[dk][:, icc * 512:(icc + 1) * 512])

            def v_spec(jt):
                return (f"v{jt}", 512,
                        lambda dk: vT[dk][:, jt * 128:(jt + 1) * 128],
                        lambda dk: wsb["wvT"][dk][:])

            def k_evac(ot, j0, ps, eng):
                jw = min(512, KLE - j0)
                evac(KT[ot][:, j0:j0 + jw], ps[:, 0:jw], eng)

            def q_evac(ot, icc, ps, eng):
                evac(QT[ot][:, icc * 512:(icc + 1) * 512], ps[:], eng)

            def v_evac(jt, ps):
                vs_out = VS[jt][:].rearrange("p (h d) -> p h d", d=HD + 1)
                nc.vector.tensor_scalar(
                    vs_out[:, :, 0:HD],
                    ps[:].rearrange("p (h d) -> p h d", d=HD),
                    m_sb[:, jt:jt + 1],
                    None,
                    mybir.AluOpType.mult,
                )
                nc.vector.tensor_copy(
                    vs_out[:, :, HD].squeeze(),
                    m_sb[:, jt:jt + 1].broadcast_to([128, H]),
                )

            def wo_group(itl, ic):
                # two query sub-chunks with interleaved accumulation chains
                tiles = []
                for half in range(2):
                    tiles.append(ppsX.tile([128, D], F32, tag="aux", name=f"ops{ic}_{itl}_{half}"))
                for hp in range(4):
                    for half, o_ps in enumerate(tiles):
                        c0 = ic * 512 + itl * 256 + half * 128
                        nc.tensor.matmul(
                            o_ps[:],
                            A2[hp][:, c0:c0 + 128],
                            wsb["woT"][hp][:],
                            start=(hp == 0),
                            stop=(hp == 3),
                        )
                for half, o_ps in enumerate(tiles):
                    c0 = ic * 512 + itl * 256 + half * 128
                    o_sb = pb.tile([128, D], F32, tag="osb", name=f"osb{ic}_{itl}_{half}")
                    nc.vector.tensor_copy(o_sb[:], o_ps[:])
                    nc.sync.dma_start(out_d[c0:c0 + 128, :], o_sb[:])

            # ---- prefix: all Q(ic0) projections run during the kT DMA
            # stall; K0 follows per column-arrival.  V streams in as hp0
            # loop fillers just ahead of its PV consumer.
            t = _mm_pair([q_spec(0, 0), q_spec(1, 0)])
            q_evac(0, 0, t[0], "s")
            q_evac(1, 0, t[1], "v")
            t = _mm_pair([q_spec(2, 0), q_spec(3, 0)])
            q_evac(2, 0, t[0], "s")
            q_evac(3, 0, t[1], "v")
            t = _mm_pair([k_spec(0, 0), k_spec(0, 512)])
            k_evac(0, 0, t[0], "s")
            k_evac(0, 512, t[1], "v")
            if KLE > 1024:
                t = _mm_pair([k_spec(0, 1024)])
                k_evac(0, 1024, t[0], "s")

            # filler queues of (spec, evac) pairs, popped two at a time
            filler = {0: [], 1: []}
            for jt in range(NJT):
                filler[0].append(("v", jt, 0))
            for j0 in range(0, KLE, 512):
                filler[0].append(("k", 1, j0))
            for j0 in range(0, KLE, 512):
                filler[0].append(("k", 2, j0))
            filler[0].append(("q", 1, 1))
            filler[0].append(("q", 2, 1))
            for j0 in range(0, KLE, 512):
                filler[0].append(("k", 3, j0))
            filler[0].append(("q", 3, 1))
            filler[0].append(("q", 0, 1))
            for itl in range(2):
                filler[1].append(("wo", itl, 0))

            def run_filler(ic, hp, jt):
                # V groups must stay ahead of their PV consumer; K/Q groups
                # for ot must land before head-pair ot starts
                pair = []
                while filler[ic] and len(pair) < 2:
                    kind, a, b = filler[ic][0]
                    if kind in ("k", "q") and a > hp + 1:
                        break
                    if kind == "wo" and len(pair) == 1:
                        break  # wo groups emit singly (own psum + dma)
                    pair.append(filler[ic].pop(0))
                    if pair[0][0] == "wo":
                        break
                if not pair:
                    return
                if pair[0][0] == "wo":
                    wo_group(pair[0][1], pair[0][2])
                    return
                specs = []
                for kind, a, b in pair:
                    specs.append(k_spec(a, b) if kind == "k" else
                                 q_spec(a, b) if kind == "q" else v_spec(a))
                ts = _mm_pair(specs)
                for (kind, a, b), ps in zip(pair, ts):
                    if kind == "k":
                        k_evac(a, b, ps, "v")
                    elif kind == "q":
                        q_evac(a, b, ps, "v")
                    else:
                        v_evac(a, ps)

            # ---- attention ----------------------------------------------
            for ic in range(2):
                i0 = ic * 512
                den_g = pr.tile([8, 512], F32, tag="deng", name=f"deng{ic}")
                pv_tiles = []
                for hp in range(4):
                    he, ho = 2 * hp, 2 * hp + 1
                    pv_e = ppsPV.tile([65, 512], F32, tag="pv", name=f"pve{hp}_{ic}")
                    pv_o = ppsPV.tile([65, 512], F32, tag="pv", name=f"pvo{hp}_{ic}")

                    def s_mm(jt):
                        s_ps = ppsS.tile([128, 1024], F32, tag="s", name=f"s{hp}_{ic}_{jt}")
                        for po2, sl in ((0, slice(0, 512)), (HD, slice(512, 1024))):
                            nc.tensor.matmul(
                                s_ps[:, sl],
                                KT[hp][po2:po2 + HD, jt * 128:(jt + 1) * 128],
                                QT[hp][po2:po2 + HD, i0:i0 + 512],
                                start=True, stop=True,
                            )
                        return s_ps

                    s_cur = s_mm(0)
                    for jt in range(NJT):
                        e_t = pe_pool.tile([128, 1024], BF16, tag="e", name=f"e{hp}_{ic}_{jt}")
                        nc.scalar.activation(e_t[:], s_cur[:], EXP, scale=SCALE, bias=ebias[:, 0:1])
                        if jt + 1 < NJT:
                            s_cur = s_mm(jt + 1)
                        if ic == 0 and hp == 0:
                            run_filler(ic, hp, jt)
                        nc.tensor.matmul(
                            pv_e[0:65, :],
                            VS[jt][:, he * (HD + 1):(he + 1) * (HD + 1)],
                            e_t[:, 0:512],
                            start=(jt == 0), stop=(jt == NJT - 1),
                        )
                        nc.tensor.matmul(
                            pv_o[0:65, :],
                            VS[jt][:, ho * (HD + 1):(ho + 1) * (HD + 1)],
                            e_t[:, 512:1024],
                            start=(jt == 0), stop=(jt == NJT - 1),
                        )
                        if not (ic == 0 and hp == 0) and jt % 2 == 1:
                            run_filler(ic, hp, jt)
                    # evacuate raw pair + gather denominators; frees pv slots
                    nc.vector.tensor_copy(A2r[hp][0:HD, i0:i0 + 512], pv_e[0:HD, :])
                    nc.vector.tensor_copy(A2r[hp][HD:128, i0:i0 + 512], pv_o[0:HD, :])
                    # DMA can't read PSUM: bounce den rows through partition 64
                    # of an SBUF stage, then cross-partition SBUF->SBUF DMA
                    stage = pb.tile([65, 1024], F32, tag="dstage", name=f"dst{hp}_{ic}")
                    nc.vector.tensor_copy(stage[64:65, 0:512], pv_e[64:65, :])
                    nc.vector.tensor_copy(stage[64:65, 512:1024], pv_o[64:65, :])
                    nc.sync.dma_start(den_g[he:he + 1, :], stage[64:65, 0:512])
                    nc.sync.dma_start(den_g[ho:ho + 1, :], stage[64:65, 512:1024])
                    run_filler(ic, hp, -1)

                # batched normalization for this query chunk
                ln_g = pr.tile([8, 512], F32, tag="lng", name=f"lng{ic}")
                r_g = pr.tile([8, 512], F32R, tag="rg", name=f"rg{ic}")
                nc.scalar.activation(ln_g[:], den_g[:], LN)
                nc.scalar.activation(r_g[:], ln_g[:], EXP, scale=-1.0)
                for hp in range(4):
                    bc = ppsX.tile([128, 512], F32, tag="aux", name=f"bc{hp}_{ic}")
                    nc.tensor.matmul(
                        bc[:],
                        sel[:, hp * 128:(hp + 1) * 128],
                        r_g[:],
                        start=True, stop=True,
                    )
                    nc.vector.tensor_mul(
                        A2[hp][:, i0:i0 + 512], A2r[hp][:, i0:i0 + 512], bc[:]
                    )
            # tail: second-chunk output projection
            for itl in range(2):
                wo_group(itl, 1)

    return nc


_NC_CACHE = {}


def _get_nc(KLE):
    if KLE not in _NC_CACHE:
        nc = build_kernel(KLE)
        _legalize_waits(nc)
        _NC_CACHE[KLE] = nc
    return _NC_CACHE[KLE]


def shard_inputs(query, key, value, Wq, Wk, Wv, Wo, attn_mask):
    idxs = [np.nonzero(np.asarray(attn_mask[b]) != 0)[0] for b in range(B)]
    maxcnt = max((len(ix) for ix in idxs), default=1)
    KLE = max(128, -(-maxcnt // 128) * 128)
    wqT = np.ascontiguousarray(np.asarray(Wq, np.float32).T)
    wkT = np.ascontiguousarray(np.asarray(Wk, np.float32).T)
    wvT = np.ascontiguousarray(np.asarray(Wv, np.float32).T.astype(_BF16NP))
    woT = np.ascontiguousarray(np.asarray(Wo, np.float32).T)
    in_maps = []
    for c in range(8):
        b, half = c // 2, c % 2
        idx = idxs[b]
        kc = np.zeros((D, KLE), np.float32)
        vc = np.zeros((D, KLE), _BF16NP)
        kc[:, : len(idx)] = np.asarray(key[b], np.float32)[idx].T
        vc[:, : len(idx)] = np.asarray(value[b], np.float32)[idx].T.astype(_BF16NP)
        mf = np.zeros(KLE, np.float32)
        mf[: len(idx)] = 1.0
        in_maps.append({
            "qT": np.ascontiguousarray(
                np.asarray(query[b, half * QS:(half + 1) * QS], np.float32).T
            ),
            "kT": kc,
            "vT": vc,
            "wqT": wqT, "wkT": wkT, "wvT": wvT, "woT": woT,
            "mask2d": np.ascontiguousarray(mf.reshape(KLE // 128, 128).T),
        })
    return in_maps, KLE


def kernel(query, key, value, Wq, Wk, Wv, Wo, attn_mask, _trace=False, _trace_kwargs=None):
    from concourse.bass_utils import run_bass_kernel_spmd

    in_maps, KLE = shard_inputs(query, key, value, Wq, Wk, Wv, Wo, attn_mask)
    nc = _get_nc(KLE)
    res = run_bass_kernel_spmd(
        nc, in_maps, list(range(8)), trace=_trace, **(_trace_kwargs or {})
    )
    out = np.empty((B, Q, D), dtype=np.float32)
    for c in range(8):
        b, half = c // 2, c % 2
        out[b, half * QS:(half + 1) * QS] = res.results[c]["out"]
    if _trace:
        kernel._last_results = res
    return out
